# revision 1
# baseline (speedup 1.0000x reference)
"""Distributed full-hidden attention on 8 trn2 NeuronCores.

Math (per reference):
  q = x @ Wq.T + bq ; k, v likewise          [B, S, H]
  scores = q @ k.T / sqrt(64)                [B, S, S]  (full hidden dim)
  out = softmax(scores) @ v @ Wo.T + bo      [B, S, H]

Weight folding (host side): with M = Wq.T @ Wk and N = Wv.T @ Wo.T,
  scores = x M x.T + (x Wq.T bk).1^T + 1.(x Wk.T bq)^T + const
The query-side bias term is constant along keys, so softmax drops it;
the key-side term c = x @ (Wk.T bq) survives as an exp bias. Similarly
  softmax(.) @ v @ Wo.T + bo = (1/d) (E @ x) @ N + (Wo bv + bo)
with E the unnormalized exp weights and d its row sums. So the kernel
needs NO k/v/o projections and NO collectives: each core gets the full
x (bf16, host-replicated), computes g = x_local @ M (fp32r), the two
big bf16 matmuls E = exp(x g.T/8) and U = E.T-style accumulation, then
U @ N. Sequence-parallel: core r owns queries x[:, r*512:(r+1)*512, :].

Precision: g-proj and out-proj on the PE in fp32r (full rate, ~1.5e-4
matmul rel err); the two big attention matmuls in bf16 (end-to-end rel
err ~6e-3 vs the fp32 reference).

Layouts per core (t = b*512 + s_local, 1024 local tokens; tau = global
token b*4096 + s):
  xT   [H, T]    fp32r (host-transposed local shard)   -> g-proj lhs
  xtf  [H, B*S]  bf16  (full x, h-major)               -> scores lhsT
  xf   [B*S, H]  bf16  (full x, natural)               -> attn lhsT
  gT[o, t] = M.T-as-lhsT proj of xT
  scoresT[sk, tq] = xtf.T @ gT ; exp on ACT (scale=1/8, bias=c/8)
  d[1, tq] = running gpsimd partition-reduce of the exp tiles (keeps
      the softmax denominator entirely off the PE and DVE)
  attnUT[h, tq]: hidden dim split 4+4 over two passes, each half
      accumulated directly in PSUM over all 32 key chunks (start/stop
      spanning the batch; 4 attn banks + 2-3 scores banks + spare <= 8),
      pass B re-reads the SBUF-resident exp tiles; pass A emits the
      attn matmuls skewed one chunk behind the scores so the exp
      latency hides under the next scores group (96% PE occupancy)
  out[t, o] = (attnUT.T @ N) * (1/d) + (Wo bv + bo)
"""

import numpy as np
import ml_dtypes

import concourse.mybir as mybir
import concourse.tile as tile
from concourse import bacc
from concourse.bass_utils import run_bass_kernel_spmd

N_CORES = 8
B, S, H = 2, 4096, 1024
S_LOC = S // N_CORES      # 512 tokens per batch per core
T = B * S_LOC             # 1024 local tokens
P = 128
HC = H // P               # 8 chunks of the hidden dim
NKC = S // P              # 32 key chunks of 128 per batch
NKG = S // 512            # 8 key groups of 512 per batch
F32 = mybir.dt.float32
F32R = mybir.dt.float32r
BF16 = mybir.dt.bfloat16
AF = mybir.ActivationFunctionType
ALU = mybir.AluOpType
SCALE = 1.0 / 8.0         # 1/sqrt(HEAD_DIM=64)

_CACHE = {}


def build_program():
    nc = bacc.Bacc(
        "TRN2",
        target_bir_lowering=False,
        debug=False,
        enable_asserts=False,
        num_devices=N_CORES,
    )

    xT_d = nc.dram_tensor("xT", [H, T], F32R, kind="ExternalInput").ap()
    m_d = nc.dram_tensor("m_mat", [H, H], F32R, kind="ExternalInput").ap()
    n_d = nc.dram_tensor("n_mat", [H, H], F32R, kind="ExternalInput").ap()
    xtf_d = nc.dram_tensor("xtf", [H, B * S], BF16, kind="ExternalInput").ap()
    xf_d = nc.dram_tensor("xf", [B * S, H], BF16, kind="ExternalInput").ap()
    c8_d = nc.dram_tensor("c8", [B * S], F32, kind="ExternalInput").ap()
    bout_d = nc.dram_tensor("b_out", [H], F32, kind="ExternalInput").ap()
    out_d = nc.dram_tensor("out", [T, H], F32, kind="ExternalOutput").ap()
    # fused-load views: [partition, row-chunk, col]
    xtf_v = xtf_d.rearrange("(oc p) t -> p oc t", p=P)
    xf_v = xf_d.rearrange("(g p) h -> p g h", p=P)

    with tile.TileContext(nc) as tc:
        with (
            tc.tile_pool(name="psum", bufs=8, space="PSUM") as pp,
            tc.tile_pool(name="misc", bufs=1) as pmisc,
            tc.tile_pool(name="gTp", bufs=1) as pgT,
        ):
            # --- small constants / biases -------------------------------
            ones1 = pmisc.tile([1, P], F32, name="ones1")
            nc.vector.memset(ones1[:], 1.0)
            onescol = pmisc.tile([P, 1], BF16, name="onescol")
            nc.vector.memset(onescol[:], 1.0)

            # c/8 per key token: [128, 64] (col = global key chunk)
            c8_sb = pmisc.tile([P, B * S // P], F32, name="c8_sb")
            nc.sync.dma_start(c8_sb[:], c8_d.rearrange("(a p) -> p a", p=P))

            bout_row = pmisc.tile([1, H], F32, name="bout_row")
            nc.sync.dma_start(bout_row[:], bout_d.rearrange("(o h) -> o h", o=1))
            bout_bc = pmisc.tile([P, H], F32, name="bout_bc")
            for c2 in range(2):
                ps = pp.tile([P, 512], F32, name="ps_b", tag="bank")
                nc.tensor.matmul(
                    ps[:], ones1[:], bout_row[:, c2 * 512:(c2 + 1) * 512],
                    start=True, stop=True,
                )
                nc.scalar.activation(
                    bout_bc[:, c2 * 512:(c2 + 1) * 512], ps[:], AF.Copy
                )

            gT_sb = [pgT.tile([P, T], BF16, name=f"gT{h}") for h in range(HC)]
            dT_sb = pmisc.tile([P, B * 4], F32, name="dT_sb")
            dtmp_sb = pmisc.tile([1, S_LOC], F32, name="dtmp_sb")
            dacc_sb = pmisc.tile([1, S_LOC], F32, name="dacc_sb")

            # key/value/exp streaming pools opened early so the first key
            # group's loads (which depend only on kernel inputs) can be
            # prefetched during the g-projection
            with (
                tc.tile_pool(name="ktp", bufs=3) as pKT,
                tc.tile_pool(name="expp", bufs=33) as pexp,
                tc.tile_pool(name="vstp", bufs=6) as pV,
            ):
              # =========== phase A: g = x_local @ M  (gT layout) ==========
              with (
                tc.tile_pool(name="xTp", bufs=1) as pxT,
                tc.tile_pool(name="wp", bufs=8) as pw,
              ):
                # issue the first-half columns of M and xT before the second
                # halves so the first psum group's operands land in ~4MB
                xT_sb = [pxT.tile([P, T], F32R, name=f"xT{h}")
                         for h in range(HC)]
                m_sb = [pw.tile([P, H], F32R, name="w", tag="w")
                        for _ in range(HC)]
                for h in range(HC):
                    nc.sync.dma_start(
                        m_sb[h][:, 0:P], m_d[h * P:(h + 1) * P, 0:P]
                    )
                for h in range(HC):
                    nc.sync.dma_start(
                        xT_sb[h][:, 0:512], xT_d[h * P:(h + 1) * P, 0:512]
                    )
                for h in range(HC):
                    nc.sync.dma_start(
                        m_sb[h][:, P:512], m_d[h * P:(h + 1) * P, P:512]
                    )
                for h in range(HC):
                    nc.sync.dma_start(
                        m_sb[h][:, 512:1024], m_d[h * P:(h + 1) * P, 512:1024]
                    )
                for h in range(HC):
                    nc.sync.dma_start(
                        xT_sb[h][:, 512:1024],
                        xT_d[h * P:(h + 1) * P, 512:1024]
                    )
                # prefetch key group (b=0, kg=0) behind the phase-A loads
                pre_kt = pKT.tile([P, HC * 512], BF16, name="kt", tag="kt")
                nc.sync.dma_start(pre_kt[:], xtf_v[:, :, 0:512])
                pre_vt = pV.tile([P, 4 * 512], BF16, name="vst", tag="vst")
                nc.sync.dma_start(pre_vt[:], xf_v[:, 0:4, 0:512])
                for tc2 in range(2):
                    for oc in range(HC):
                        ps = pp.tile([P, 512], F32, name="ps_p", tag="bank")
                        for h in range(HC):
                            nc.tensor.matmul(
                                ps[:],
                                m_sb[h][:, oc * P:(oc + 1) * P],
                                xT_sb[h][:, tc2 * 512:(tc2 + 1) * 512],
                                start=(h == 0), stop=(h == HC - 1),
                            )
                        nc.scalar.activation(
                            gT_sb[oc][:, tc2 * 512:(tc2 + 1) * 512],
                            ps[:], AF.Copy,
                        )

              # =========== phase B: attention =============================
              with (
                tc.tile_pool(name="attp", bufs=1) as pattn,
                tc.tile_pool(name="wop", bufs=1) as pwo,
                tc.tile_pool(name="outp", bufs=4) as pout,
                tc.tile_pool(name="rdp", bufs=2) as prd,
              ):
                attnT = [pattn.tile([P, T], F32R, name=f"attnT{h}")
                         for h in range(HC)]
                n_sb = []
                for h in range(HC):
                    w_t = pwo.tile([P, H], F32R, name=f"wo{h}")
                    nc.sync.dma_start(w_t[:], n_d[h * P:(h + 1) * P, :])
                    n_sb.append(w_t)

                for b in range(B):
                    cb = slice(b * S_LOC, (b + 1) * S_LOC)
                    # pass A: scores + exp + d + attn for h 0..511, with the
                    # attn halves accumulated directly in PSUM over all keys
                    paA = [pp.tile([P, S_LOC], F32, name=f"paA{h2}",
                                   tag="bank") for h2 in range(4)]
                    es_all = []
                    pend = None
                    for kg in range(NKG):
                        tau0 = b * S + kg * 512
                        g0 = b * 32 + kg * 4
                        if b == 0 and kg == 0:
                            kt_all, vt_all = pre_kt, pre_vt
                        else:
                            kt_all = pKT.tile([P, HC * 512], BF16, name="kt",
                                              tag="kt")
                            nc.sync.dma_start(
                                kt_all[:], xtf_v[:, :, tau0:tau0 + 512]
                            )
                            vt_all = pV.tile([P, 4 * 512], BF16, name="vst",
                                             tag="vst")
                            nc.sync.dma_start(
                                vt_all[:], xf_v[:, g0:g0 + 4, 0:512]
                            )
                        for j in range(4):
                            kc = kg * 4 + j
                            ps_s = pp.tile([P, S_LOC], F32, name="ps_s",
                                           tag="bank")
                            for oc in range(HC):
                                nc.tensor.matmul(
                                    ps_s[:],
                                    kt_all[:, oc * 512 + j * P:
                                           oc * 512 + (j + 1) * P],
                                    gT_sb[oc][:, cb],
                                    start=(oc == 0), stop=(oc == HC - 1),
                                )
                            e_t = pexp.tile([P, S_LOC], BF16, name="exp",
                                            tag="exp")
                            nc.scalar.activation(
                                e_t[:], ps_s[:], AF.Exp, scale=SCALE,
                                bias=c8_sb[:, b * 32 + kc: b * 32 + kc + 1],
                            )
                            es_all.append(e_t)
                            if kc == 0:
                                nc.gpsimd.tensor_reduce(
                                    dacc_sb[:], e_t[:],
                                    axis=mybir.AxisListType.C, op=ALU.add,
                                )
                            else:
                                nc.gpsimd.tensor_reduce(
                                    dtmp_sb[:], e_t[:],
                                    axis=mybir.AxisListType.C, op=ALU.add,
                                )
                                nc.gpsimd.tensor_add(
                                    dacc_sb[:], dtmp_sb[:], dacc_sb[:]
                                )
                            # one-chunk skew: emit attn for the previous
                            # chunk so the exp of this one hides under it
                            if pend is not None:
                                pkc, pj, pvt, pe_t = pend
                                for h2 in range(4):
                                    nc.tensor.matmul(
                                        paA[h2][:],
                                        pvt[:, pj * 512 + h2 * P:
                                            pj * 512 + (h2 + 1) * P],
                                        pe_t[:],
                                        start=(pkc == 0), stop=False,
                                    )
                            pend = (kc, j, vt_all, e_t)
                    pkc, pj, pvt, pe_t = pend
                    for h2 in range(4):
                        nc.tensor.matmul(
                            paA[h2][:],
                            pvt[:, pj * 512 + h2 * P:
                                pj * 512 + (h2 + 1) * P],
                            pe_t[:],
                            start=False, stop=True,
                        )
                    for h2 in range(4):
                        nc.scalar.activation(attnT[h2][:, cb], paA[h2][:],
                                             AF.Copy)

                    # pass B: attn for h 512..1023, re-reading the exp tiles
                    paB = [pp.tile([P, S_LOC], F32, name=f"paB{h2}",
                                   tag="bank") for h2 in range(4)]
                    for kg in range(NKG):
                        g0 = b * 32 + kg * 4
                        vtB = pV.tile([P, 4 * 512], BF16, name="vst",
                                      tag="vst")
                        nc.sync.dma_start(
                            vtB[:], xf_v[:, g0:g0 + 4, 512:1024]
                        )
                        for j in range(4):
                            kc = kg * 4 + j
                            for h2 in range(4):
                                nc.tensor.matmul(
                                    paB[h2][:],
                                    vtB[:, j * 512 + h2 * P:
                                        j * 512 + (h2 + 1) * P],
                                    es_all[kc][:],
                                    start=(kc == 0), stop=(kc == NKC - 1),
                                )
                    for h2 in range(4):
                        nc.scalar.activation(attnT[4 + h2][:, cb], paB[h2][:],
                                             AF.Copy)
                    d_sb = prd.tile([1, S_LOC], F32, name="d_sb", tag="rd")
                    nc.vector.reciprocal(d_sb[:], dacc_sb[:])
                    for tt in range(4):
                        nc.sync.dma_start(
                            dT_sb[:, b * 4 + tt: b * 4 + tt + 1],
                            d_sb[0:1, tt * P:(tt + 1) * P],
                        )

                    # ---- output projection for this batch's tokens ----
                    for ts in range(b * 4, b * 4 + 4):
                        for oc2 in range(2):
                            po = pp.tile([P, 512], F32, name="po", tag="bank")
                            for h2 in range(HC):
                                nc.tensor.matmul(
                                    po[:],
                                    attnT[h2][:, ts * P:(ts + 1) * P],
                                    n_sb[h2][:, oc2 * 512:(oc2 + 1) * 512],
                                    start=(h2 == 0), stop=(h2 == HC - 1),
                                )
                            o_t = pout.tile([P, 512], F32, name="ot", tag="ot")
                            nc.vector.scalar_tensor_tensor(
                                o_t[:], po[:], dT_sb[:, ts:ts + 1],
                                bout_bc[:, oc2 * 512:(oc2 + 1) * 512],
                                ALU.mult, ALU.add,
                            )
                            nc.sync.dma_start(
                                out_d[ts * P:(ts + 1) * P,
                                      oc2 * 512:(oc2 + 1) * 512],
                                o_t[:],
                            )

    nc.compile()
    return nc


def make_in_maps(x, Wq, bq, Wk, bk, Wv, bv, Wo, bo):
    x = np.asarray(x, np.float32)
    Wq = np.asarray(Wq, np.float32)
    Wk = np.asarray(Wk, np.float32)
    Wv = np.asarray(Wv, np.float32)
    Wo = np.asarray(Wo, np.float32)
    bq = np.asarray(bq, np.float32)
    bk = np.asarray(bk, np.float32)
    bv = np.asarray(bv, np.float32)
    bo = np.asarray(bo, np.float32)

    m_mat = np.ascontiguousarray((Wq.T @ Wk))                 # [h_in, h_in2]
    n_mat = np.ascontiguousarray(Wv.T @ Wo.T)                 # [h_in, o]
    xfull = x.reshape(B * S, H)                               # tau-major
    xf = xfull.astype(ml_dtypes.bfloat16)
    xtf = np.ascontiguousarray(xfull.T).astype(ml_dtypes.bfloat16)
    c8 = (xfull @ (Wk.T @ bq)) * np.float32(SCALE)            # key-side bias
    b_out = Wo @ bv + bo

    common = dict(m_mat=m_mat, n_mat=n_mat, xtf=xtf, xf=xf,
                  c8=c8.astype(np.float32), b_out=b_out.astype(np.float32))
    in_maps = []
    for r in range(N_CORES):
        xr = x[:, r * S_LOC:(r + 1) * S_LOC, :].reshape(T, H)
        in_maps.append(dict(xT=np.ascontiguousarray(xr.T), **common))
    return in_maps


def assemble(results):
    shards = np.stack([res["out"] for res in results])      # [R, T, H]
    return np.ascontiguousarray(
        shards.reshape(N_CORES, B, S_LOC, H).transpose(1, 0, 2, 3)
        .reshape(B, S, H)
    )


def kernel(x, Wq, bq, Wk, bk, Wv, bv, Wo, bo):
    if "nc" not in _CACHE:
        _CACHE["nc"] = build_program()
    nc = _CACHE["nc"]
    in_maps = make_in_maps(x, Wq, bq, Wk, bk, Wv, bv, Wo, bo)
    try:
        res = run_bass_kernel_spmd(nc, in_maps, core_ids=list(range(N_CORES)))
    except Exception:
        # the exec unit occasionally reports a transient unrecoverable
        # state (NRT_EXEC_UNIT_UNRECOVERABLE); one retry has always
        # succeeded
        import time
        time.sleep(15)
        res = run_bass_kernel_spmd(nc, in_maps, core_ids=list(range(N_CORES)))
    return assemble(res.results)



# revision 2
# speedup vs baseline: 1.1940x; 1.1940x over previous
"""Distributed full-hidden attention on 8 trn2 NeuronCores — fp8 DoubleRow.

Math (per reference):
  q = x @ Wq.T + bq ; k, v likewise          [B, S, H]
  scores = q @ k.T / sqrt(64)                [B, S, S]  (full hidden dim)
  out = softmax(scores) @ v @ Wo.T + bo      [B, S, H]

Weight folding (host): M = Wq.T @ Wk, N = Wv.T @ Wo.T. The query-side
bias drops inside softmax; the key-side bias c = x @ (Wk.T bq) survives
as an exp bias; out = softmax(x M x.T / 8 + c) @ x @ N + (Wo bv + bo).
Sequence-parallel SPMD: core r owns queries x[:, r*512:(r+1)*512, :];
no collectives (x replicated host-side).

All four matmuls run on the PE as fp8e4 DoubleRow (0.5 cycles/row,
256-deep contraction pairs = 4x bf16 throughput in the cost model),
using double-fp8 3-term products to keep bf16-level accuracy:
  a @ b ~= a_hi @ b_hi + a_hi @ b_lo + a_lo @ b_hi
with hi = fp8(v), lo = fp8(v - hi). Operands whose scale sits in e4m3's
subnormal range (M, N ~ N(0, 1/1024)) are pre-scaled by 32 host-side
(descaled via the exp scale / output STT) so the lo residual stays
representable.

Softmax without max-subtraction: exp goes to bf16 (huge range), the
per-query normalizer d is accumulated as a DVE partial-sum tile and
folded BEFORE quantization: p = (64/d) * E_wide via one gpsimd
partition_all_reduce (broadcast result) + DVE reciprocal + Pool mult.
The fp8 attn weights are therefore already normalized (x64 for
subnormal headroom), the attention matmul needs no final divide, and a
single 8-PSUM-bank pass over key-chunk pairs covers the whole hidden
dim (no second value pass, no v reload).

Per-core PE cycles @2.4GHz: g-proj 49k + scores 2x98k + attn 2x98k +
out-proj 2x25k ~= 492k cycles ~= 205 us (vs 273 us bf16 baseline).
"""

import numpy as np
import ml_dtypes

import concourse.mybir as mybir
import concourse.tile as tile
from concourse import bacc, bass_isa
from concourse.bass_utils import run_bass_kernel_spmd

N_CORES = 8
B, S, H = 2, 4096, 1024
S_LOC = S // N_CORES      # 512 tokens per batch per core
T = B * S_LOC             # 1024 local tokens
P = 128
HC = H // P               # 8 hidden chunks
HPAIR = HC // 2           # 4 hidden-chunk pairs
NKC = S // P              # 32 key chunks per batch
NKP = NKC // 2            # 16 key-chunk pairs per batch
NKG = S // 512            # 8 key groups of 512 per batch
F32 = mybir.dt.float32
BF16 = mybir.dt.bfloat16
FP16 = mybir.dt.float16
FP8 = mybir.dt.float8e4
AF = mybir.ActivationFunctionType
ALU = mybir.AluOpType
DR = mybir.MatmulPerfMode.DoubleRow
MS = 32.0                 # M pre-scale (keeps fp8 residual normal-range)
NS = 32.0                 # N pre-scale
PS = 64.0                 # attn-weight pre-scale (subnormal headroom)
SCALE = 1.0 / (8.0 * MS)  # exp scale: 1/sqrt(64) and undo MS

F8NP = ml_dtypes.float8_e4m3fn

_CACHE = {}


def build_program():
    nc = bacc.Bacc(
        "TRN2",
        target_bir_lowering=False,
        debug=False,
        enable_asserts=False,
        num_devices=N_CORES,
    )

    xtp_hi_d = nc.dram_tensor("xtp_hi", [4 * P, 2 * T], FP8,
                              kind="ExternalInput").ap()
    xtp_lo_d = nc.dram_tensor("xtp_lo", [4 * P, 2 * T], FP8,
                              kind="ExternalInput").ap()
    mp_hi_d = nc.dram_tensor("mp_hi", [4 * P, 2 * H], FP8,
                             kind="ExternalInput").ap()
    mp_lo_d = nc.dram_tensor("mp_lo", [4 * P, 2 * H], FP8,
                             kind="ExternalInput").ap()
    np_hi_d = nc.dram_tensor("np_hi", [4 * P, 2 * H], FP8,
                             kind="ExternalInput").ap()
    np_lo_d = nc.dram_tensor("np_lo", [4 * P, 2 * H], FP8,
                             kind="ExternalInput").ap()
    ktf_hi_d = nc.dram_tensor("ktf_hi", [H, B * S], FP8,
                              kind="ExternalInput").ap()
    ktf_lo_d = nc.dram_tensor("ktf_lo", [H, B * S], FP8,
                              kind="ExternalInput").ap()
    vf_hi_d = nc.dram_tensor("vf_hi", [B * S, H], FP8,
                             kind="ExternalInput").ap()
    vf_lo_d = nc.dram_tensor("vf_lo", [B * S, H], FP8,
                             kind="ExternalInput").ap()
    c8_d = nc.dram_tensor("c8", [B * S], F32, kind="ExternalInput").ap()
    bout_d = nc.dram_tensor("b_out", [H], F32, kind="ExternalInput").ap()
    out_d = nc.dram_tensor("out", [T, H], F32, kind="ExternalOutput").ap()

    kt_hi_v = ktf_hi_d.rearrange("(oc p) t -> p oc t", p=P)
    kt_lo_v = ktf_lo_d.rearrange("(oc p) t -> p oc t", p=P)
    vf_hi_v = vf_hi_d.rearrange("(g p) h -> p g h", p=P)
    vf_lo_v = vf_lo_d.rearrange("(g p) h -> p g h", p=P)

    with tile.TileContext(nc) as tc:
        with (
            tc.tile_pool(name="psum", bufs=8, space="PSUM") as pp,
            tc.tile_pool(name="misc", bufs=1) as pmisc,
            tc.tile_pool(name="gTp", bufs=1) as pgT,
        ):
            # --- small constants / biases -------------------------------
            ones1 = pmisc.tile([1, P], F32, name="ones1")
            nc.vector.memset(ones1[:], 1.0)
            s_inv64 = pmisc.tile([P, 1], F32, name="s_inv64")
            nc.vector.memset(s_inv64[:], 1.0 / 64.0)
            s_invPS = pmisc.tile([P, 1], F32, name="s_invPS")
            nc.vector.memset(s_invPS[:], 1.0 / PS)
            s_invNS = pmisc.tile([P, 1], F32, name="s_invNS")
            nc.vector.memset(s_invNS[:], 1.0 / NS)

            c8_sb = pmisc.tile([P, B * S // P], F32, name="c8_sb")
            nc.sync.dma_start(c8_sb[:], c8_d.rearrange("(a p) -> p a", p=P))

            bout_row = pmisc.tile([1, H], F32, name="bout_row")
            nc.sync.dma_start(bout_row[:], bout_d.rearrange("(o h) -> o h",
                                                            o=1))
            bout_bc = pmisc.tile([P, H], F32, name="bout_bc")
            for c2 in range(2):
                ps = pp.tile([P, 512], F32, name="ps_b", tag="bank")
                nc.tensor.matmul(
                    ps[:], ones1[:], bout_row[:, c2 * 512:(c2 + 1) * 512],
                    start=True, stop=True,
                )
                nc.scalar.activation(
                    bout_bc[:, c2 * 512:(c2 + 1) * 512], ps[:], AF.Copy
                )

            # g pairs: pair c holds h-chunks (2c, 2c+1); free = [2, T]
            gH = [pgT.tile([P, 2 * T], FP8, name=f"gH{c}")
                  for c in range(HPAIR)]
            gL = [pgT.tile([P, 2 * T], FP8, name=f"gL{c}")
                  for c in range(HPAIR)]
            dpart = pmisc.tile([P, S_LOC], F32, name="dpart")
            dall = pmisc.tile([P, S_LOC], F32, name="dall")
            dinv = pmisc.tile([P, S_LOC], F32, name="dinv")
            # attn pairs (fp8 hi/lo), free = [2, T]
            aH = [pgT.tile([P, 2 * T], FP8, name=f"aH{c}")
                  for c in range(HPAIR)]
            aL = [pgT.tile([P, 2 * T], FP8, name=f"aL{c}")
                  for c in range(HPAIR)]
            # N pairs, free = [2, H]
            npH = [pgT.tile([P, 2 * H], FP8, name=f"npH{c}")
                   for c in range(HPAIR)]
            npL = [pgT.tile([P, 2 * H], FP8, name=f"npL{c}")
                   for c in range(HPAIR)]

            with (
                tc.tile_pool(name="ktp", bufs=3) as pKT,
                tc.tile_pool(name="vstp", bufs=3) as pV,
                tc.tile_pool(name="ewp", bufs=33) as pEW,
                tc.tile_pool(name="p16p", bufs=4) as pP16,
                tc.tile_pool(name="ehp", bufs=4) as pEh,
                tc.tile_pool(name="outp", bufs=4) as pout,
            ):
              # =========== phase A: g' = x_local @ (M*32), fp8 pairs =====
              with (
                tc.tile_pool(name="xTp", bufs=1) as pxT,
                tc.tile_pool(name="wp", bufs=1) as pw,
              ):
                mh_sb = [pw.tile([P, 2 * H], FP8, name=f"mh{c}")
                         for c in range(HPAIR)]
                ml_sb = [pw.tile([P, 2 * H], FP8, name=f"ml{c}")
                         for c in range(HPAIR)]
                xh_sb = [pxT.tile([P, 2 * T], FP8, name=f"xh{c}")
                         for c in range(HPAIR)]
                xl_sb = [pxT.tile([P, 2 * T], FP8, name=f"xl{c}")
                         for c in range(HPAIR)]
                for c in range(HPAIR):
                    nc.sync.dma_start(mh_sb[c][:],
                                      mp_hi_d[c * P:(c + 1) * P, :])
                    nc.sync.dma_start(xh_sb[c][:],
                                      xtp_hi_d[c * P:(c + 1) * P, :])
                for c in range(HPAIR):
                    nc.sync.dma_start(ml_sb[c][:],
                                      mp_lo_d[c * P:(c + 1) * P, :])
                    nc.sync.dma_start(xl_sb[c][:],
                                      xtp_lo_d[c * P:(c + 1) * P, :])
                # prefetch N pairs and first key-group tiles behind phase A
                for c in range(HPAIR):
                    nc.sync.dma_start(npH[c][:],
                                      np_hi_d[c * P:(c + 1) * P, :])
                    nc.sync.dma_start(npL[c][:],
                                      np_lo_d[c * P:(c + 1) * P, :])
                pre_ktH = pKT.tile([P, HC, 512], FP8, name="ktH", tag="ktH")
                nc.sync.dma_start(pre_ktH[:], kt_hi_v[:, :, 0:512])
                pre_ktL = pKT.tile([P, HC, 512], FP8, name="ktL", tag="ktL")
                nc.sync.dma_start(pre_ktL[:], kt_lo_v[:, :, 0:512])

                for tc2 in range(2):
                    ts_ = slice(tc2 * 512, (tc2 + 1) * 512)
                    for oc in range(HC):
                        ps = pp.tile([P, 512], F32, name="ps_p", tag="bank")
                        for c in range(HPAIR):
                            mh_v = mh_sb[c][:].rearrange(
                                "p (two h) -> p two h", two=2
                            )[:, :, oc * P:(oc + 1) * P]
                            ml_v = ml_sb[c][:].rearrange(
                                "p (two h) -> p two h", two=2
                            )[:, :, oc * P:(oc + 1) * P]
                            xh_v = xh_sb[c][:].rearrange(
                                "p (two t) -> p two t", two=2
                            )[:, :, ts_]
                            xl_v = xl_sb[c][:].rearrange(
                                "p (two t) -> p two t", two=2
                            )[:, :, ts_]
                            nc.tensor.matmul(ps[:], mh_v, xh_v,
                                             start=(c == 0), stop=False,
                                             perf_mode=DR)
                            nc.tensor.matmul(ps[:], mh_v, xl_v,
                                             start=False, stop=False,
                                             perf_mode=DR)
                            nc.tensor.matmul(ps[:], ml_v, xh_v,
                                             start=False,
                                             stop=(c == HPAIR - 1),
                                             perf_mode=DR)
                        cg, half = divmod(oc, 2)
                        dst = slice(half * T + tc2 * 512,
                                    half * T + (tc2 + 1) * 512)
                        nc.scalar.activation(gH[cg][:, dst], ps[:], AF.Copy)
                        nc.vector.tensor_sub(gL[cg][:, dst], ps[:],
                                             gH[cg][:, dst])

              # =========== phase B: attention ============================
              for b in range(B):
                cb = slice(b * S_LOC, (b + 1) * S_LOC)
                ew_ts = []
                # ---- sweep 1: scores + exp(bf16) + d partial sums ------
                for kg in range(NKG):
                    tau0 = b * S + kg * 512
                    if b == 0 and kg == 0:
                        ktH_t, ktL_t = pre_ktH, pre_ktL
                    else:
                        ktH_t = pKT.tile([P, HC, 512], FP8, name="ktH",
                                         tag="ktH")
                        nc.sync.dma_start(ktH_t[:],
                                          kt_hi_v[:, :, tau0:tau0 + 512])
                        ktL_t = pKT.tile([P, HC, 512], FP8, name="ktL",
                                         tag="ktL")
                        nc.sync.dma_start(ktL_t[:],
                                          kt_lo_v[:, :, tau0:tau0 + 512])
                    for j in range(4):
                        kc = kg * 4 + j
                        js = slice(j * P, (j + 1) * P)
                        ps_s = pp.tile([P, S_LOC], F32, name="ps_s",
                                       tag="bank")
                        for c in range(HPAIR):
                            ktH_v = ktH_t[:, 2 * c:2 * c + 2, js]
                            ktL_v = ktL_t[:, 2 * c:2 * c + 2, js]
                            gH_v = gH[c][:].rearrange(
                                "p (two t) -> p two t", two=2)[:, :, cb]
                            gL_v = gL[c][:].rearrange(
                                "p (two t) -> p two t", two=2)[:, :, cb]
                            nc.tensor.matmul(ps_s[:], ktH_v, gH_v,
                                             start=(c == 0), stop=False,
                                             perf_mode=DR)
                            nc.tensor.matmul(ps_s[:], ktL_v, gH_v,
                                             start=False, stop=False,
                                             perf_mode=DR)
                            nc.tensor.matmul(ps_s[:], ktH_v, gL_v,
                                             start=False,
                                             stop=(c == HPAIR - 1),
                                             perf_mode=DR)
                        ew = pEW.tile([P, S_LOC], BF16, name="ew", tag="ew")
                        nc.scalar.activation(
                            ew[:], ps_s[:], AF.Exp, scale=SCALE,
                            bias=c8_sb[:, b * 32 + kc:b * 32 + kc + 1],
                        )
                        ew_ts.append(ew)
                        if kc == 0:
                            nc.vector.tensor_copy(dpart[:], ew[:])
                        else:
                            nc.vector.tensor_add(dpart[:], dpart[:], ew[:])

                # ---- d finalize: dinv = 64/d broadcast over partitions --
                nc.gpsimd.partition_all_reduce(dall[:], dpart[:], P,
                                               bass_isa.ReduceOp.add)
                nc.vector.tensor_scalar_mul(dall[:], dall[:], s_inv64[:])
                nc.vector.reciprocal(dinv[:], dall[:])

                # ---- sweep 2: normalize+quantize, single-pass attn ------
                paA = [pp.tile([P, S_LOC], F32, name=f"paA{h2}", tag="bank")
                       for h2 in range(HC)]
                vt_bufs = {}
                for jp in range(NKP):
                    kgA, posA = divmod(jp, 2)
                    if posA == 0:
                        g0 = b * 32 + kgA * 4
                        vtH_t = pV.tile([P, 4, H], FP8, name="vtH",
                                        tag="vtH")
                        nc.sync.dma_start(vtH_t[:],
                                          vf_hi_v[:, g0:g0 + 4, :])
                        vtL_t = pV.tile([P, 4, H], FP8, name="vtL",
                                        tag="vtL")
                        nc.sync.dma_start(vtL_t[:],
                                          vf_lo_v[:, g0:g0 + 4, :])
                        vt_bufs[kgA] = (vtH_t, vtL_t)
                    vtH_t, vtL_t = vt_bufs[kgA]
                    EhT = pEh.tile([P, 2, S_LOC], FP8, name="EhT", tag="EhT")
                    ElT = pEh.tile([P, 2, S_LOC], FP8, name="ElT", tag="ElT")
                    for c2 in range(2):
                        kc = 2 * jp + c2
                        p16 = pP16.tile([P, S_LOC], FP16, name="p16",
                                        tag="p16")
                        nc.gpsimd.tensor_tensor(p16[:], ew_ts[kc][:],
                                                dinv[:], ALU.mult)
                        nc.scalar.activation(EhT[:, c2, :], p16[:], AF.Copy)
                        nc.vector.tensor_sub(ElT[:, c2, :], p16[:],
                                             EhT[:, c2, :])
                    for h2 in range(HC):
                        hs = slice(h2 * P, (h2 + 1) * P)
                        vH_v = vtH_t[:, 2 * posA:2 * posA + 2, hs]
                        vL_v = vtL_t[:, 2 * posA:2 * posA + 2, hs]
                        nc.tensor.matmul(paA[h2][:], vH_v, EhT[:],
                                         start=(jp == 0), stop=False,
                                         perf_mode=DR)
                        nc.tensor.matmul(paA[h2][:], vH_v, ElT[:],
                                         start=False, stop=False,
                                         perf_mode=DR)
                        nc.tensor.matmul(paA[h2][:], vL_v, EhT[:],
                                         start=False, stop=(jp == NKP - 1),
                                         perf_mode=DR)

                # ---- attn psum -> fp8 hi/lo pairs (descale by 1/PS) -----
                for h2 in range(HC):
                    cg, half = divmod(h2, 2)
                    dst = slice(half * T + b * S_LOC,
                                half * T + (b + 1) * S_LOC)
                    nc.scalar.activation(aH[cg][:, dst], paA[h2][:],
                                         AF.Copy, scale=1.0 / PS)
                    nc.vector.scalar_tensor_tensor(
                        aL[cg][:, dst], paA[h2][:], s_invPS[:],
                        aH[cg][:, dst], ALU.mult, ALU.subtract,
                    )

                # ---- out projection for this batch's tokens -------------
                for ts in range(4):
                    tok = slice(b * S_LOC + ts * P, b * S_LOC + (ts + 1) * P)
                    for oc2 in range(2):
                        os_ = slice(oc2 * 512, (oc2 + 1) * 512)
                        po = pp.tile([P, 512], F32, name="po", tag="bank")
                        for c in range(HPAIR):
                            aH_v = aH[c][:].rearrange(
                                "p (two t) -> p two t", two=2)[:, :, tok]
                            aL_v = aL[c][:].rearrange(
                                "p (two t) -> p two t", two=2)[:, :, tok]
                            nH_v = npH[c][:].rearrange(
                                "p (two o) -> p two o", two=2)[:, :, os_]
                            nL_v = npL[c][:].rearrange(
                                "p (two o) -> p two o", two=2)[:, :, os_]
                            nc.tensor.matmul(po[:], aH_v, nH_v,
                                             start=(c == 0), stop=False,
                                             perf_mode=DR)
                            nc.tensor.matmul(po[:], aH_v, nL_v,
                                             start=False, stop=False,
                                             perf_mode=DR)
                            nc.tensor.matmul(po[:], aL_v, nH_v,
                                             start=False,
                                             stop=(c == HPAIR - 1),
                                             perf_mode=DR)
                        o_t = pout.tile([P, 512], F32, name="ot", tag="ot")
                        nc.vector.scalar_tensor_tensor(
                            o_t[:], po[:], s_invNS[:], bout_bc[:, os_],
                            ALU.mult, ALU.add,
                        )
                        nc.sync.dma_start(
                            out_d[(b * 4 + ts) * P:(b * 4 + ts + 1) * P, os_],
                            o_t[:],
                        )

    nc.compile()
    return nc


def _pair_pack(w):
    """[4*P rows, C cols] h-pair layout from [H, C]: row = c*128+p holds
    chunk-pair c; cols = [2, C] (chunk 2c then 2c+1)."""
    hh, cc = w.shape
    return np.ascontiguousarray(
        w.reshape(HPAIR, 2, P, cc).transpose(0, 2, 1, 3).reshape(4 * P, 2 * cc)
    )


def _split8(a):
    hi = a.astype(F8NP)
    lo = (a - hi.astype(np.float32)).astype(F8NP)
    return hi, lo


def make_in_maps(x, Wq, bq, Wk, bk, Wv, bv, Wo, bo):
    x = np.asarray(x, np.float32)
    Wq = np.asarray(Wq, np.float32)
    Wk = np.asarray(Wk, np.float32)
    Wv = np.asarray(Wv, np.float32)
    Wo = np.asarray(Wo, np.float32)
    bq = np.asarray(bq, np.float32)
    bv = np.asarray(bv, np.float32)
    bo = np.asarray(bo, np.float32)

    m_s = (Wq.T @ Wk) * np.float32(MS)
    n_s = (Wv.T @ Wo.T) * np.float32(NS)
    mh, ml = _split8(m_s)
    nh, nl = _split8(n_s)
    mp_hi, mp_lo = _pair_pack(mh), _pair_pack(ml)
    np_hi, np_lo = _pair_pack(nh), _pair_pack(nl)

    xfull = x.reshape(B * S, H)
    vf_hi, vf_lo = _split8(xfull)
    ktf_hi = np.ascontiguousarray(vf_hi.T)
    ktf_lo = np.ascontiguousarray(vf_lo.T)
    c8 = ((xfull @ (Wk.T @ bq)) * np.float32(0.125)).astype(np.float32)
    b_out = (Wo @ bv + bo).astype(np.float32)

    common = dict(mp_hi=mp_hi, mp_lo=mp_lo, np_hi=np_hi, np_lo=np_lo,
                  ktf_hi=ktf_hi, ktf_lo=ktf_lo, vf_hi=vf_hi, vf_lo=vf_lo,
                  c8=c8, b_out=b_out)
    in_maps = []
    for r in range(N_CORES):
        xr = x[:, r * S_LOC:(r + 1) * S_LOC, :].reshape(T, H)
        xth, xtl = _split8(np.ascontiguousarray(xr.T))
        in_maps.append(dict(xtp_hi=_pair_pack(xth), xtp_lo=_pair_pack(xtl),
                            **common))
    return in_maps


def assemble(results):
    shards = np.stack([res["out"] for res in results])      # [R, T, H]
    return np.ascontiguousarray(
        shards.reshape(N_CORES, B, S_LOC, H).transpose(1, 0, 2, 3)
        .reshape(B, S, H)
    )


def kernel(x, Wq, bq, Wk, bk, Wv, bv, Wo, bo):
    if "nc" not in _CACHE:
        _CACHE["nc"] = build_program()
    nc = _CACHE["nc"]
    in_maps = make_in_maps(x, Wq, bq, Wk, bk, Wv, bv, Wo, bo)
    try:
        res = run_bass_kernel_spmd(nc, in_maps, core_ids=list(range(N_CORES)))
    except Exception:
        # the exec unit occasionally reports a transient unrecoverable
        # state (NRT_EXEC_UNIT_UNRECOVERABLE); one retry has always
        # succeeded
        import time
        time.sleep(15)
        res = run_bass_kernel_spmd(nc, in_maps, core_ids=list(range(N_CORES)))
    return assemble(res.results)
